# revision 6
# baseline (speedup 1.0000x reference)
"""MuSc (Mutual Scoring) Trainium2 kernel.

Problem: nn_BatchMuSc — Z:[16,1369,1024] patch features, cls_tokens:[16,1024].
MSM: for each image i, per-patch score = mean of the 4 smallest per-image
min-distances (excluding self). Then image scores -> min-max norm -> MMO over
cls-token similarity.

Strategy (8 NeuronCores, data-parallel over query image pairs):
  - Core c owns query images (2c, 2c+1). All inputs to core c are ROTATED so
    position 0 = image 2c; self-exclusion positions are then core-invariant
    (pos 0 for local img 0, pos 1 for local img 1) => one SPMD program.
  - Host pre-transposes Z to feature-major fp16 tiles [128, 8k, 1408] per
    image (refs padded 1369->1408 with a constant vector, whose distance is
    always huge) and pre-broadcasts ref squared-norms across partitions.
  - Device, per (query image, 128-query block, ref position, 512-ref chunk):
    PSUM[q,r] = sum_k (-2*q_k)*r_k via 8 fp16 matmuls; one fused DVE
    tensor_tensor_reduce adds ref norms, min-reduces over the chunk and
    chains the running min across chunks => m[q, pos] = min d^2 - |q|^2.
  - Tail on device: 4 smallest of m row via iterative masked min; each
    + |q|^2 -> sqrt (ACT); mean -> per-patch score. Host does the tiny
    [16]-vector min-max norm + 16x16 MMO tail in float64.
"""

import os
import numpy as np

N = 16            # images
L = 1369          # patches per image
C = 1024          # feature dim
NCORES = 8
LP = 1408         # padded patches (11 * 128)
NQB = 11          # query blocks of 128
KCH = 8           # contraction chunks of 128
CHUNKS = [(0, 512), (512, 512), (1024, 384)]
PAD_VAL = np.float16(2.0)   # pad-row feature value; pad d^2 ~ |q|^2+4096-4*sum(q) >> real min
PAD_NORM = 4096.0           # C * PAD_VAL^2
BIG = 3.0e38

_CACHE = {}


def _build():
    import concourse.bacc as bacc
    import concourse.tile as tile
    from concourse import mybir

    f16 = mybir.dt.float16
    f32 = mybir.dt.float32
    Sqrt = mybir.ActivationFunctionType.Sqrt
    Alu = mybir.AluOpType
    AxX = mybir.AxisListType.X

    nc = bacc.Bacc("TRN2", target_bir_lowering=False, debug=False)

    zt = nc.dram_tensor("zt", [N, 128, KCH, LP], f16, kind="ExternalInput").ap()
    qt = nc.dram_tensor("qt", [2, 128, KCH, LP], f16, kind="ExternalInput").ap()
    nb = nc.dram_tensor("nb", [N, 128, LP], f32, kind="ExternalInput").ap()
    q2 = nc.dram_tensor("q2", [2, 128, NQB], f32, kind="ExternalInput").ap()
    out = nc.dram_tensor("scores", [2, 128, NQB], f32, kind="ExternalOutput").ap()

    with tile.TileContext(nc) as tc:
        with (
            tc.tile_pool(name="qpool", bufs=1) as qpool,
            tc.tile_pool(name="refpool", bufs=2) as refpool,
            tc.tile_pool(name="nbpool", bufs=2) as nbpool,
            tc.tile_pool(name="mpool", bufs=1) as mpool,
            tc.tile_pool(name="smpool", bufs=8) as smpool,
            tc.tile_pool(name="scrpool", bufs=4) as scrpool,
            tc.tile_pool(name="scorepool", bufs=1) as scorepool,
            tc.tile_pool(name="psum", bufs=6, space="PSUM") as psum,
        ):
            # resident query lhsT tiles (already scaled by -2 on host)
            qsb = []
            for i in range(2):
                t = qpool.tile([128, KCH, LP], f16, name=f"q{i}", tag=f"q{i}")
                nc.sync.dma_start(t[:], qt[i])
                qsb.append(t)
            q2sb = []
            for i in range(2):
                t = qpool.tile([128, NQB], f32, name=f"q2_{i}", tag=f"q2_{i}")
                nc.sync.dma_start(t[:], q2[i])
                q2sb.append(t)

            # persistent min accumulators m[i][qb] : [128, N] (d^2 - |q|^2 per ref pos)
            msb = [[mpool.tile([128, N], f32, name=f"m_{i}_{qb}", tag=f"m_{i}_{qb}") for qb in range(NQB)]
                   for i in range(2)]
            for i in range(2):
                for qb in range(NQB):
                    nc.vector.memset(msb[i][qb][:], BIG)

            scoresb = [scorepool.tile([128, NQB], f32, name=f"sc{i}", tag=f"sc{i}") for i in range(2)]

            for t in range(N):
                rsb = refpool.tile([128, KCH, LP], f16, name="ref", tag="ref")
                nc.sync.dma_start(rsb[:], zt[t])
                nbt = nbpool.tile([128, LP], f32, name="nbt", tag="nbt")
                nc.sync.dma_start(nbt[:], nb[t])

                for i in range(2):
                    if t == i:   # self image: skip
                        continue
                    for qb in range(NQB):
                        prev = None
                        for ci, (r0, w) in enumerate(CHUNKS):
                            pt = psum.tile([128, 512], f32, name="qr", tag="qr")
                            for k in range(KCH):
                                nc.tensor.matmul(
                                    pt[:, :w],
                                    lhsT=qsb[i][:, k, qb * 128:(qb + 1) * 128],
                                    rhs=rsb[:, k, r0:r0 + w],
                                    start=(k == 0),
                                    stop=(k == KCH - 1),
                                )
                            scr = scrpool.tile([128, 512], f32, name="scr", tag="scr")
                            nc.vector.tensor_tensor(
                                scr[:, :w], pt[:, :w], nbt[:, r0:r0 + w], op=Alu.add)
                            cm = smpool.tile([128, 1], f32, name="cmin", tag="cmin")
                            nc.vector.tensor_reduce(
                                cm[:], scr[:, :w], axis=AxX, op=Alu.min)
                            if ci == 0:
                                prev = cm
                            elif ci < len(CHUNKS) - 1:
                                nx = smpool.tile([128, 1], f32, name="nx", tag="nx")
                                nc.vector.tensor_tensor(
                                    nx[:], prev[:], cm[:], op=Alu.min)
                                prev = nx
                            else:
                                nc.vector.tensor_tensor(
                                    msb[i][qb][:, t:t + 1], prev[:], cm[:],
                                    op=Alu.min)

            # tail: per (img, qblock) extract 4 smallest, sqrt(x+|q|^2), mean
            for i in range(2):
                for qb in range(NQB):
                    m = msb[i][qb]
                    dsum = None
                    for it in range(4):
                        rmin = smpool.tile([128, 1], f32, name="rmin", tag="rmin")
                        nc.vector.tensor_reduce(rmin[:], m[:], axis=AxX, op=Alu.min)
                        if it < 3:
                            mask = smpool.tile([128, N], f32, name="mask", tag="mask")
                            nc.vector.tensor_scalar(
                                out=mask[:], in0=m[:],
                                scalar1=rmin[:], scalar2=BIG,
                                op0=Alu.is_equal, op1=Alu.mult,
                            )
                            nc.vector.tensor_tensor(m[:], m[:], mask[:], op=Alu.add)
                        d = smpool.tile([128, 1], f32, name="dist", tag="dist")
                        nc.scalar.activation(d[:], rmin[:], Sqrt,
                                             bias=q2sb[i][:, qb:qb + 1], scale=1.0)
                        if dsum is None:
                            dsum = d
                        else:
                            s = smpool.tile([128, 1], f32, name="dsum", tag="dsum")
                            nc.vector.tensor_add(s[:], dsum[:], d[:])
                            dsum = s
                    nc.vector.tensor_scalar_mul(
                        scoresb[i][:, qb:qb + 1], dsum[:], 0.25)

            for i in range(2):
                nc.sync.dma_start(out[i], scoresb[i][:])
    nc.compile()
    return nc


def _host_prep(Z):
    Zp = np.full((N, LP, C), PAD_VAL, dtype=np.float16)
    Zp[:, :L, :] = Z.astype(np.float16)
    # [j, p, k, r] = Zp[j, r, 128k+p]
    zt_all = np.ascontiguousarray(Zp.reshape(N, LP, KCH, 128).transpose(0, 3, 2, 1))
    nr = (Z.astype(np.float64) ** 2).sum(-1)
    nrp = np.full((N, LP), PAD_NORM)
    nrp[:, :L] = nr
    nrp = nrp.astype(np.float32)
    return zt_all, nrp


def kernel(Z, cls_tokens):
    import concourse.bass_utils as bass_utils

    Z = np.asarray(Z)
    cls_tokens = np.asarray(cls_tokens)

    if "nc" not in _CACHE:
        _CACHE["nc"] = _build()
    nc = _CACHE["nc"]

    zt_all, nrp = _host_prep(Z)

    in_maps = []
    for c in range(NCORES):
        order = [(2 * c + t) % N for t in range(N)]
        zt_core = np.ascontiguousarray(zt_all[order])
        qt_core = zt_all[2 * c:2 * c + 2] * np.float16(-2.0)
        nb_core = np.ascontiguousarray(
            np.broadcast_to(nrp[order][:, None, :], (N, 128, LP)))
        q2_core = np.ascontiguousarray(
            nrp[2 * c:2 * c + 2].reshape(2, NQB, 128).transpose(0, 2, 1))
        in_maps.append({"zt": zt_core, "qt": qt_core,
                        "nb": nb_core, "q2": q2_core})

    trace = bool(int(os.environ.get("KERNEL_TRACE", "0")))
    res = bass_utils.run_bass_kernel_spmd(
        nc, in_maps, core_ids=list(range(NCORES)), trace=trace)
    _CACHE["last_results"] = res

    patch_scores = np.zeros((N, L), dtype=np.float64)
    for c in range(NCORES):
        sc = res.results[c]["scores"]          # [2, 128, NQB]
        flat = sc.transpose(0, 2, 1).reshape(2, LP)   # [2, qb*128+p]
        patch_scores[2 * c:2 * c + 2] = flat[:, :L]

    # ---- tiny tail on host (float64) ----
    img = patch_scores.max(-1)
    s = (img - img.min()) / (img.max() - img.min())
    W = cls_tokens.astype(np.float64) @ cls_tokens.astype(np.float64).T
    outs = []
    for k in (1, 2, 3):
        thr = np.sort(W, axis=-1)[:, N - k][:, None]
        Wm = np.where(W >= thr, W, 0.0)
        P = Wm / Wm.sum(-1, keepdims=True)
        outs.append(P @ s)
    return np.stack(outs, -1).mean(-1).astype(np.float32)
